# revision 1
# baseline (speedup 1.0000x reference)
"""CollapseLoss kernel for Trainium2, 8-way row-sharded.

Reference computation (N=16384 rows, D=128):
    x_n   = row-normalize(feature_clusters)            # F.normalize(dim=1)
    d[i]  = dot(x_n[i+1], x_n[i])        i = 0..N-2
    out   = (d + 1/(N-1))**2

Sharding: 2048 rows per core. Host-side sharding materializes, per core, the
SBUF image xc[128, 17*128]: partition p holds rows 16p..16p+15 of the shard
(blocks 0..15) followed by row 16(p+1) (block 16 — the t=15 partner row,
which for p=127 is the next shard's first row: the halo).  Every device-side
load is then a plain contiguous column-range DMA, and the consecutive-row
dot for (16p+t, 16p+t+1) is a free-dim-shifted product:
    S[:, t] = sum_j AB[:, t*128+j] * AB[:, t*128+128+j]
(the t=15 partner sits right after block 15, so `in1` ranges stay
contiguous across the whole tile).

Work split (default = coarse-grained; knobs in CFG allow the fine-grained
alternative):
  DVE : bulk shifted-product tensor_tensor per group + segmented reduce
        -> S; segmented reduce of the ACT squares -> NE; finals chain
  ACT : one Square pass per group (the last one also covers block 16)

On real TRN2 (measured with K-repeat NEFFs through the PJRT path)
per-instruction fixed costs are ~3-4x the instruction cost model's, so
~18 large instructions beat ~75 fused per-block ones (~28us vs ~44us
steady-state per iteration), even though the cost model prefers the
fine-grained split (12.6us vs 16us modeled).

Tail avoids the inaccurate-rsqrt problem algebraically:
    (S/sqrt(m) + c)^2 == (S + c*sqrt(m))^2 * (1/m)   with m = na*nb
where sqrt(m) only scales the tiny c-term (c ~ 6.1e-5), so ACT's
loose-budget Sqrt table is ample (an integer-magic bitcast sqrt is also
available via act_sqrt=False), and 1/m is the exact DVE reciprocal.  The
ACT activation table is loaded at t~0 via a dummy activation on a const AP
so it never blocks the stream.
"""

import sys
import numpy as np
from contextlib import ExitStack

try:
    import concourse  # noqa: F401
except ImportError:  # grading env without the sitecustomize path
    for _p in ("/opt/trn_rl_repo", "/root/.axon_site/_ro/trn_rl_repo"):
        if _p not in sys.path:
            sys.path.append(_p)

N_ROWS = 16384
D = 128
N_CORES = 8
R = N_ROWS // N_CORES  # 2048 rows per core
P = 128                # partitions
Q = R // P             # 16 row-blocks per partition
C_CONST = 1.0 / (N_ROWS - 1)
SQRT_MAGIC = 0x1FBD1DF5  # bitcast(i>>1 + magic) ~= sqrt, rel err <= 4.5%

# tuning knobs.  The default is the coarse-grained variant: on real TRN2
# hardware (measured via K-repeat NEFFs) per-instruction fixed costs are
# ~3-4x the cost model's, so ~14 big instructions (3 pipelined loads, one
# bulk product+reduce, one Square pass+reduce over all 17 blocks, finals,
# store) beat ~75 fused small ones (~28us vs ~44us steady-state per
# iteration) even though the instruction cost model prefers the
# fine-grained split.
CFG = {
    # input DMA ranges in block units (block 16 = halo/partner), load order
    "load_order": ((0, 6), (6, 12), (12, 17)),
    # compute groups (products/norms emitted per group, in this order)
    "groups": ((0, 16),),
    "pool_groups": (),            # group indices: products via Pool TT
    "bulk_groups": (0,),          # group indices: products via DVE TT+reduce
    "acc_blocks": (),             # norms via ACT Square+accum
    "stt_norm_blocks": (),        # norms via DVE STT (in0=in1=blk)
    "halo_in_pass": True,         # fold block-16 norms into the last pass
    "finals_groups": ((0, 16),),  # [start, end) output block ranges
    "act_sqrt": True,             # ACT table Sqrt vs DVE int-magic sqrt
    "dummy_square": True,         # hoist the ACT table load to t~0
}

_CACHE = {}


def _build_nc(cfg=None, repeat=1):
    import concourse.bacc as bacc
    import concourse.tile as tile
    from concourse import mybir

    cfg = dict(CFG, **(cfg or {}))
    f32 = mybir.dt.float32
    AF = mybir.ActivationFunctionType
    ALU = mybir.AluOpType
    acc_blocks = set(cfg["acc_blocks"])
    stt_norms = set(cfg["stt_norm_blocks"])
    pool_groups = set(cfg["pool_groups"])
    bulk_groups = set(cfg.get("bulk_groups", ()))
    fgroups = cfg["finals_groups"]

    nc = bacc.Bacc(
        "TRN2",
        target_bir_lowering=False,
        debug=False,
        enable_asserts=False,
        num_devices=N_CORES,
    )
    xc = nc.dram_tensor("xc", [P, (Q + 1) * D], f32, kind="ExternalInput").ap()
    out = nc.dram_tensor("out", [R], f32, kind="ExternalOutput").ap()
    out_pq = out.rearrange("(p q) -> p q", p=P)

    with tile.TileContext(nc) as tc:
        with ExitStack() as ctx:
            data = ctx.enter_context(tc.tile_pool(name="data", bufs=1))
            scr = ctx.enter_context(tc.tile_pool(name="scr", bufs=6))
            stat = ctx.enter_context(tc.tile_pool(name="stat", bufs=1))

            # `repeat` exists only for wall-clock benchmarking: the whole
            # pipeline body K times in one NEFF; pool-slot reuse serializes
            # successive repeats like a steady-state stream.
            for _rep in range(repeat):
                AB = data.tile([P, (Q + 1) * D], f32, tag="AB", name=f"AB{_rep}")
                PR = data.tile([P, Q * D], f32)   # pool products / ACT squares
                SQ = data.tile([P, (Q + 1) * D], f32)
                S = stat.tile([P, Q], f32)        # raw consecutive-row dots
                NE = stat.tile([P, Q + 1], f32)   # squared norms incl. block 16

                if cfg["dummy_square"]:
                    # hoist the single ACT table load to t~0; with act_sqrt the
                    # dummy is a Sqrt so the set (sqrt_and_others) covers both
                    dum = scr.tile([P, 1], f32, tag="dum")
                    one = nc.const_aps.aps[(f32, 1.0)]
                    dfn = AF.Sqrt if cfg["act_sqrt"] else AF.Square
                    nc.scalar.activation(out=dum, in_=one[:P], func=dfn)

                for lo, hi in cfg["load_order"]:
                    nc.sync.dma_start(out=AB[:, lo * D:hi * D],
                                      in_=xc[:, lo * D:hi * D])

                # halo norms (block 16): ACT accum or DVE STT, unless folded
                # into the last group's Square pass (halo_in_pass)
                hb = AB[:, Q * D:(Q + 1) * D]
                if cfg.get("halo_in_pass"):
                    pass
                elif Q in stt_norms:
                    sqb = scr.tile([P, D], f32, tag="pr")
                    nc.vector.scalar_tensor_tensor(
                        out=sqb, in0=hb, scalar=1.0, in1=hb,
                        op0=ALU.bypass, op1=ALU.mult, accum_out=NE[:, Q:Q + 1])
                else:
                    sqb = scr.tile([P, D], f32, tag="sq")
                    nc.scalar.activation(out=sqb, in_=hb, func=AF.Square,
                                         accum_out=NE[:, Q:Q + 1])

                fired = set()
                done_blocks = set()
                for gidx, (ba, bb) in enumerate(cfg["groups"]):
                    lo, hi = ba * D, bb * D
                    # products (in1 spans one block past, contiguous incl. halo)
                    if gidx in pool_groups or gidx in bulk_groups:
                        peng = nc.gpsimd if gidx in pool_groups else nc.vector
                        peng.tensor_tensor(out=PR[:, lo:hi],
                                           in0=AB[:, lo:hi],
                                           in1=AB[:, lo + D:hi + D],
                                           op=ALU.mult)
                        nc.vector.tensor_reduce(
                            S[:, ba:bb],
                            PR[:, lo:hi].rearrange("p (q d) -> p q d", q=bb - ba),
                            axis=mybir.AxisListType.X, op=ALU.add)
                    else:
                        for t in range(ba, bb):
                            blk = AB[:, t * D:(t + 1) * D]
                            nxt = AB[:, (t + 1) * D:(t + 2) * D]
                            pr = scr.tile([P, D], f32, tag="pr", name=f"pr{t}")
                            nc.vector.scalar_tensor_tensor(
                                out=pr, in0=blk, scalar=1.0, in1=nxt,
                                op0=ALU.bypass, op1=ALU.mult,
                                accum_out=S[:, t:t + 1])

                    # norms
                    for t in [t for t in range(ba, bb) if t in stt_norms]:
                        blk = AB[:, t * D:(t + 1) * D]
                        sqt = scr.tile([P, D], f32, tag="pr", name=f"sqs{t}")
                        nc.vector.scalar_tensor_tensor(
                            out=sqt, in0=blk, scalar=1.0, in1=blk,
                            op0=ALU.bypass, op1=ALU.mult,
                            accum_out=NE[:, t:t + 1])
                    for t in [t for t in range(ba, bb) if t in acc_blocks]:
                        blk = AB[:, t * D:(t + 1) * D]
                        sqt = scr.tile([P, D], f32, tag="sq", name=f"sqa{t}")
                        nc.scalar.activation(out=sqt, in_=blk, func=AF.Square,
                                             accum_out=NE[:, t:t + 1])
                    run = []
                    ptl = [t for t in range(ba, bb)
                           if t not in acc_blocks and t not in stt_norms]
                    if cfg.get("halo_in_pass") and bb == Q:
                        ptl.append(Q)  # fold block 16 into the final run
                    for t in ptl + [None]:
                        if run and (t is None or t != run[-1] + 1):
                            a, b = run[0], run[-1] + 1
                            if cfg.get("sq_bulk_dve"):
                                nc.vector.tensor_tensor(
                                    out=SQ[:, a * D:b * D],
                                    in0=AB[:, a * D:b * D],
                                    in1=AB[:, a * D:b * D], op=ALU.mult)
                            else:
                                nc.scalar.activation(out=SQ[:, a * D:b * D],
                                                     in_=AB[:, a * D:b * D],
                                                     func=AF.Square)
                            nc.vector.tensor_reduce(
                                NE[:, a:b],
                                SQ[:, a * D:b * D].rearrange(
                                    "p (q d) -> p q d", q=b - a),
                                axis=mybir.AxisListType.X, op=ALU.add)
                            run = []
                        if t is not None:
                            run.append(t)

                    # finals for any output group now fully determined
                    done_blocks.update(range(ba, bb))
                    for gi, (ga, gb) in enumerate(fgroups):
                        need = gb + 1 if gb < Q else Q
                        if gi not in fired and done_blocks >= set(range(ga, need)):
                            fired.add(gi)
                            _emit_finals(nc, stat, mybir, S, NE, out_pq,
                                         ga, gb, gi, cfg)

    nc.compile()
    return nc


def _emit_finals(nc, stat, mybir, S, NE, out_pq, ga, gb, gi, cfg):
    """out[:, ga:gb] = (S + c*sqrt(m))^2 / m for block range [ga, gb)."""
    ALU = mybir.AluOpType
    f32 = mybir.dt.float32
    i32 = mybir.dt.int32
    AF = mybir.ActivationFunctionType
    w_ = gb - ga
    m = stat.tile([P, w_], f32, name=f"m{gi}")
    nc.vector.tensor_tensor(out=m, in0=NE[:, ga:gb], in1=NE[:, ga + 1:gb + 1],
                            op=ALU.mult)
    w = stat.tile([P, w_], f32, name=f"w{gi}")
    nc.vector.reciprocal(w, m)   # off the sqrt chain; joins at the end
    s0 = stat.tile([P, w_], f32, name=f"s0{gi}")
    if cfg["act_sqrt"]:
        nc.scalar.activation(out=s0, in_=m, func=AF.Sqrt)
    else:
        sh = stat.tile([P, w_], f32, name=f"sh{gi}")
        nc.vector.tensor_scalar(sh.bitcast(i32), m.bitcast(i32), 1, None,
                                ALU.logical_shift_right)
        nc.vector.tensor_scalar(s0.bitcast(i32), sh.bitcast(i32), SQRT_MAGIC,
                                None, ALU.add)
    u = stat.tile([P, w_], f32, name=f"u{gi}")
    nc.vector.scalar_tensor_tensor(out=u, in0=s0, scalar=C_CONST,
                                   in1=S[:, ga:gb], op0=ALU.mult, op1=ALU.add)
    v = stat.tile([P, w_], f32, name=f"v{gi}")
    nc.vector.tensor_tensor(out=v, in0=u, in1=u, op=ALU.mult)
    o = stat.tile([P, w_], f32, name=f"o{gi}")
    nc.vector.tensor_tensor(out=o, in0=v, in1=w, op=ALU.mult)
    nc.sync.dma_start(out=out_pq[:, ga:gb], in_=o)


def _get_nc():
    if "nc" not in _CACHE:
        _CACHE["nc"] = _build_nc()
    return _CACHE["nc"]


def make_in_maps(x: np.ndarray) -> list[dict[str, np.ndarray]]:
    """Host-side sharding: build each core's SBUF image xc[128, 2176]."""
    x = np.ascontiguousarray(np.asarray(x, dtype=np.float32))
    # pad one row (the out-of-range halo of the last core) with ones
    xp = np.concatenate([x, np.ones((1, D), dtype=np.float32)], axis=0)
    in_maps = []
    for c in range(N_CORES):
        sh = xp[c * R:c * R + R].reshape(P, Q * D)        # blocks 0..15
        halo = xp[c * R + 16 * np.arange(1, P + 1)]       # block 16
        xc = np.concatenate([sh, halo.reshape(P, D)], axis=1)
        in_maps.append({"xc": np.ascontiguousarray(xc)})
    return in_maps


def kernel(feature_clusters: np.ndarray) -> np.ndarray:
    from concourse.bass_utils import run_bass_kernel_spmd

    nc = _get_nc()
    in_maps = make_in_maps(feature_clusters)
    res = run_bass_kernel_spmd(nc, in_maps, list(range(N_CORES))).results
    full = np.concatenate([res[c]["out"] for c in range(N_CORES)])
    return full[:N_ROWS - 1].astype(np.float32)

